# revision 1
# baseline (speedup 1.0000x reference)
"""CREDA loss kernel for Trainium2 (8 NeuronCores, SPMD data-parallel).

Math: the loss needs only K^2 = exp(-2*d2/(2*sigma^2+EPS)) entries, never K.
With f = 2/(2*sigma^2+EPS):  K2[i,j] = exp(2f*G[i,j]) * exp(-f*r[i]) * exp(-f*r[j]),
G = X @ Y.T.  Every per-class reduction is a quadratic form u^T K2 v, so the
device computes L[c,j] = sum_i u[i,c] * exp(2f*G[i,j] + bias_i)  (fp8 DoubleRow
GEMM -> ScalarE exp with per-partition fp32 bias -> [128,4]x[128,512] bf16
reduction matmul into PSUM), and the host folds exp(-f*r[j]) plus the
right-hand class mask into a tiny dot.  The tt block's uncertainty weights
w_i^2 ride the fp32 bias (+2 ln w_i), so all device-side masks are exact 0/1.

Sharding: each core owns a 512-row slice of all three blocks (ss, tt, st);
both feature matrices live SBUF-resident (fp8, 8 MB); per-core partial
L[3,4,4096] outputs are summed on host.  No collectives.
"""

import numpy as np
import ml_dtypes

import concourse.bacc as bacc
import concourse.tile as tile
import concourse.mybir as mybir
from concourse.bass_utils import run_bass_kernel_spmd

# Problem constants (hardcoded per harness contract)
N = 4096            # N_S == N_T
D = 1024
C = 4
SIGMA = 32.0
EPS = 1e-8
LOG2 = float(np.log(2.0))
LAMBDA_CREDA = 1.0
LAMBDA_ENTROPY = 0.1

NCORES = 8
ROWS = N // NCORES      # 512 rows per core
IT = 128                # i-tile (PSUM partition dim)
JT = 512                # j-tile (PSUM free dim, one fp32 bank)
KC = 128                # contraction chunk (PE partition dim)
N_I = ROWS // IT        # 4
N_J = N // JT           # 8
N_K = D // KC           # 8
CP = 16                 # class dim padded to satisfy DoubleRow stride rule
TT_LIFT = 32.0          # lifts tt exp values; exactly undone on host

F_SCALE = 2.0 / (2.0 * SIGMA * SIGMA + EPS)   # exponent factor for K^2
ACT_SCALE = float(2.0 * F_SCALE)              # multiplies G inside the exp

BF16 = mybir.dt.bfloat16
FP32 = mybir.dt.float32
FP8 = mybir.dt.float8e4

_COMPILED = {}


def _build(repeat=1):
    nc = bacc.Bacc("TRN2", target_bir_lowering=False, debug=False)

    rhs_s = nc.dram_tensor("rhs_s", [D, N], FP8, kind="ExternalInput")
    rhs_t = nc.dram_tensor("rhs_t", [D, N], FP8, kind="ExternalInput")
    lhs_s = nc.dram_tensor("lhs_s", [D, ROWS], FP8, kind="ExternalInput")
    lhs_t = nc.dram_tensor("lhs_t", [D, ROWS], FP8, kind="ExternalInput")
    lm_s = nc.dram_tensor("lm_s", [IT, N_I, CP], FP8, kind="ExternalInput")
    lm_t = nc.dram_tensor("lm_t", [IT, N_I, CP], FP8, kind="ExternalInput")
    bias_s = nc.dram_tensor("bias_s", [IT, N_I], FP32, kind="ExternalInput")
    bias_t = nc.dram_tensor("bias_t", [IT, N_I], FP32, kind="ExternalInput")
    lout = nc.dram_tensor("lout", [3, C, N], FP32, kind="ExternalOutput")

    rs_ap = rhs_s.ap().rearrange("(k p) j -> p k j", p=KC)
    rt_ap = rhs_t.ap().rearrange("(k p) j -> p k j", p=KC)

    with tile.TileContext(nc) as tc:
        with (
            tc.tile_pool(name="const", bufs=1) as const,
            tc.tile_pool(name="ep", bufs=6) as epp,
            tc.tile_pool(name="stage", bufs=2) as stp,
            tc.tile_pool(name="gps", bufs=2, space="PSUM") as gps,
            tc.tile_pool(name="lps", bufs=2, space="PSUM") as lps,
        ):
            lhsS = const.tile([KC, N_K, ROWS], FP8)
            nc.sync.dma_start(out=lhsS, in_=lhs_s.ap().rearrange("(k p) i -> p k i", p=KC))
            lhsT = const.tile([KC, N_K, ROWS], FP8)
            nc.sync.dma_start(out=lhsT, in_=lhs_t.ap().rearrange("(k p) i -> p k i", p=KC))
            lmS = const.tile([IT, N_I, CP], FP8)
            nc.sync.dma_start(out=lmS, in_=lm_s.ap())
            lmT = const.tile([IT, N_I, CP], FP8)
            nc.sync.dma_start(out=lmT, in_=lm_t.ap())
            biS = const.tile([IT, N_I], FP32)
            nc.sync.dma_start(out=biS, in_=bias_s.ap())
            biT = const.tile([IT, N_I], FP32)
            nc.sync.dma_start(out=biT, in_=bias_t.ap())

            # both rhs matrices fully SBUF-resident, one tile per j-panel so
            # consumers gate on individual panel DMAs (16 x 0.5 MB)
            rsP = []
            rtP = []
            for j in range(N_J):
                rs = const.tile([KC, N_K, JT], FP8, tag=f"rs{j}")
                nc.sync.dma_start(out=rs, in_=rs_ap[:, :, j * JT:(j + 1) * JT])
                rsP.append(rs)
                rt = const.tile([KC, N_K, JT], FP8, tag=f"rt{j}")
                nc.sync.dma_start(out=rt, in_=rt_ap[:, :, j * JT:(j + 1) * JT])
                rtP.append(rt)

            blocks = (
                (lhsS, rsP, lmS, biS),   # ss
                (lhsT, rtP, lmT, biT),   # tt
                (lhsS, rtP, lmS, biS),   # st
            )

            def body():
                # two j-tiles per pass share one 2-bank PSUM tile and one exp;
                # all 24 reduce-MMs of a j-pair run after all 12 G+exp units,
                # so the exp tail never starves PE
                for jp in range(N_J // 2):
                    j0, j1 = 2 * jp, 2 * jp + 1
                    eps_all = []
                    for b in range(3):
                        lhs, rP, lm, bi = blocks[b]
                        eps = []
                        for q in range(N_I // 2):
                            ep2 = epp.tile([IT, 2, 2 * JT], FP8,
                                           tag=f"ep{b}_{q}", bufs=2,
                                           name=f"ep{b}_{q}_{jp}")
                            for h in range(2):
                                it = 2 * q + h
                                gp = gps.tile([IT, 2 * JT], FP32,
                                              tag="gp", bufs=3)
                                _g_group(nc, gp[:, 0:JT], lhs, rP[j0], it)
                                _g_group(nc, gp[:, JT:2 * JT], lhs, rP[j1], it)
                                nc.scalar.activation(
                                    ep2[:, h, :], gp,
                                    mybir.ActivationFunctionType.Exp,
                                    bias=bi[:, it:it + 1], scale=ACT_SCALE,
                                )
                            eps.append(ep2)
                        eps_all.append(eps)
                    for b in range(3):
                        _emit_reduce(nc, stp, lps, lout, blocks[b][2],
                                     eps_all[b], b, j0, j1)

            if repeat == 1:
                body()
            else:
                with tc.For_i(0, repeat, 1):
                    body()

    nc.compile()
    return nc


def _g_group(nc, gp, lhs, rhs, it):
    """G = X_i . Y_j over the 1024-d contraction (fp8 DoubleRow, K=256/MM)."""
    for k2 in range(N_K // 2):
        nc.tensor.matmul(
            gp,
            lhs[:, 2 * k2:2 * k2 + 2, it * IT:(it + 1) * IT],
            rhs[:, 2 * k2:2 * k2 + 2, :],
            start=(k2 == 0),
            stop=(k2 == N_K // 2 - 1),
            perf_mode=mybir.MatmulPerfMode.DoubleRow,
        )


def _emit_reduce(nc, stp, lps, lout, lm, eps, b, j0, j1):
    """fp8 DoubleRow reduce: each MM contracts an i-tile pair (K=256)."""
    nq = N_I // 2
    lp0 = lps.tile([CP, JT], FP32, tag="lp0", bufs=1, name=f"lp0_{b}_{j0}")
    lp1 = lps.tile([CP, JT], FP32, tag="lp1", bufs=1, name=f"lp1_{b}_{j0}")
    for jh, lp in ((0, lp0), (1, lp1)):
        for q in range(nq):
            nc.tensor.matmul(
                lp,
                lm[:, 2 * q:2 * q + 2, :],
                eps[q][:, :, jh * JT:(jh + 1) * JT],
                start=(q == 0), stop=(q == nq - 1),
                perf_mode=mybir.MatmulPerfMode.DoubleRow,
            )
    st = stp.tile([C, 2 * JT], FP32, name=f"st_{b}_{j0}")
    nc.vector.tensor_copy(st[:, 0:JT], lp0[0:C, :])
    nc.vector.tensor_copy(st[:, JT:2 * JT], lp1[0:C, :])
    nc.sync.dma_start(out=lout.ap()[b, :, j0 * JT:(j1 + 1) * JT], in_=st)


def _get_nc(repeat=1):
    if repeat not in _COMPILED:
        _COMPILED[repeat] = _build(repeat)
    return _COMPILED[repeat]


def _host_prep(features_s, logits_s, features_t, logits_t, labels_s):
    fs = np.asarray(features_s, dtype=np.float32)
    ft = np.asarray(features_t, dtype=np.float32)
    lt = np.asarray(logits_t, dtype=np.float32)
    lab = np.asarray(labels_s).astype(np.int64)

    # target softmax / pseudo labels / uncertainty weights (host, fp64)
    z = lt.astype(np.float64)
    z = z - z.max(axis=1, keepdims=True)
    pt = np.exp(z)
    pt /= pt.sum(axis=1, keepdims=True)
    pseudo = np.argmax(pt, axis=1)
    h2p = -np.log(np.sum(pt * pt, axis=1) + EPS) / LOG2
    h2max = np.log(float(C)) / LOG2
    w = 1.0 - h2p / (h2max + EPS)

    ms = np.zeros((N, C), dtype=np.float64)
    ms[np.arange(N), lab] = 1.0
    mt = np.zeros((N, C), dtype=np.float64)
    mt[np.arange(N), pseudo] = 1.0
    wt2 = mt * (w * w)[:, None]          # [N, C] == Wt2.T of the reference

    rs = np.sum(fs.astype(np.float64) ** 2, axis=1)
    rt = np.sum(ft.astype(np.float64) ** 2, axis=1)
    es = np.exp(-F_SCALE * rs)
    et = np.exp(-F_SCALE * rt)

    # tt-block row weights w^2 (and a xTT_LIFT lift) ride the fp32 ACT bias:
    # exp(2fG - f*rt_i + 2 ln w_i + ln TT_LIFT)
    logw2 = np.where(w > 0, 2.0 * np.log(np.maximum(w, 1e-300)), -1e30)
    bias_tt = -F_SCALE * rt + logw2 + np.log(TT_LIFT)

    xsT = np.ascontiguousarray(fs.T).astype(ml_dtypes.float8_e4m3)
    xtT = np.ascontiguousarray(ft.T).astype(ml_dtypes.float8_e4m3)

    msp = np.zeros((N, CP)); msp[:, :C] = ms
    mtp = np.zeros((N, CP)); mtp[:, :C] = mt

    in_maps = []
    for c in range(NCORES):
        sl = slice(c * ROWS, (c + 1) * ROWS)
        lm_s_c = msp[sl].reshape(N_I, IT, CP).transpose(1, 0, 2)
        lm_t_c = mtp[sl].reshape(N_I, IT, CP).transpose(1, 0, 2)
        bi_s_c = (-F_SCALE * rs[sl]).reshape(N_I, IT).T
        bi_t_c = bias_tt[sl].reshape(N_I, IT).T
        in_maps.append({
            "rhs_s": xsT,
            "rhs_t": xtT,
            "lhs_s": np.ascontiguousarray(xsT[:, sl]),
            "lhs_t": np.ascontiguousarray(xtT[:, sl]),
            "lm_s": np.ascontiguousarray(lm_s_c).astype(ml_dtypes.float8_e4m3),
            "lm_t": np.ascontiguousarray(lm_t_c).astype(ml_dtypes.float8_e4m3),
            "bias_s": np.ascontiguousarray(bi_s_c).astype(np.float32),
            "bias_t": np.ascontiguousarray(bi_t_c).astype(np.float32),
        })

    S = 512
    xs8 = xsT[:, :S].astype(np.float64).T      # [S, D] dequantized fp8
    xt8 = xtT[:, :S].astype(np.float64).T
    f8 = lambda a: a.astype(ml_dtypes.float8_e4m3).astype(np.float64)

    def _calib(Xrow, Ycol, brow):
        E = np.exp(ACT_SCALE * (Xrow @ Ycol.T) + brow[:, None])
        return float(f8(E).sum() / E.sum())

    cal = np.array([
        _calib(xs8, xs8, -F_SCALE * rs[:S]),
        _calib(xt8, xt8, bias_tt[:S]),
        _calib(xs8, xt8, -F_SCALE * rs[:S]),
    ])

    aux = dict(ms=ms, mt=mt, wt2=wt2, es=es, et=et, w=w, lab=lab, pt=pt,
               cal=cal)
    return in_maps, aux


def _host_finish(L, aux, logits_s):
    ms, mt, wt2 = aux["ms"], aux["mt"], aux["wt2"]
    es, et, lab, pt = aux["es"], aux["et"], aux["lab"], aux["pt"]

    # right-hand side masks with the column exp factor folded in
    rm_ss = ms * es[:, None]             # [N, C]
    rm_tt = wt2 * et[:, None]
    rm_st = mt * et[:, None]

    cal = aux["cal"]
    ss_s = np.einsum("cj,jc->c", L[0], rm_ss) / cal[0]
    ss_t = np.einsum("cj,jc->c", L[1], rm_tt) / TT_LIFT / cal[1]
    ss_st = np.einsum("cj,jc->c", L[2], rm_st) / cal[2]

    n_s = ms.sum(axis=0)
    n_t = mt.sum(axis=0)
    tr_s = n_s
    tr_t = wt2.sum(axis=0)

    def h2(tr, sumsq):
        info = sumsq / (tr + EPS) ** 2
        return -np.log(info + EPS) / LOG2

    h_s = h2(tr_s, ss_s)
    h_t = h2(tr_t, ss_t)
    h_mix = h2(tr_s + tr_t, ss_s + 2.0 * ss_st + ss_t)
    per_class = h_mix - 0.5 * (h_s + h_t)
    valid = (n_s >= 2) & (n_t >= 2)
    n_valid = float(valid.sum())
    creda_sum = float(np.where(valid, per_class, 0.0).sum())
    loss_creda = creda_sum / max(n_valid, 1.0) if n_valid > 0 else 0.0

    # source cross entropy (host, fp64)
    zs = np.asarray(logits_s, dtype=np.float64)
    zs = zs - zs.max(axis=1, keepdims=True)
    lse = np.log(np.exp(zs).sum(axis=1))
    logp = zs - lse[:, None]
    loss_cls = -float(np.mean(logp[np.arange(N), lab]))

    # target entropy
    loss_ent = -float(np.mean(np.sum(pt * np.log(pt + EPS), axis=1)))

    total = loss_cls + LAMBDA_CREDA * loss_creda + LAMBDA_ENTROPY * loss_ent
    return np.array(total, dtype=np.float32)


def run(inputs, trace=False, repeat=1):
    """Full pipeline; returns (loss, BassKernelResults)."""
    in_maps, aux = _host_prep(**inputs)
    nc = _get_nc(repeat)
    res = run_bass_kernel_spmd(
        nc, in_maps, core_ids=list(range(NCORES)), trace=trace,
    )
    L = np.zeros((3, C, N), dtype=np.float64)
    for r in res.results:
        L += r["lout"].astype(np.float64)
    loss = _host_finish(L, aux, inputs["logits_s"])
    return loss, res


def kernel(**inputs) -> np.ndarray:
    loss, _ = run(inputs, trace=False)
    return loss



# revision 4
# speedup vs baseline: 1.7495x; 1.7495x over previous
"""CREDA loss kernel for Trainium2 (8 NeuronCores, SPMD, symmetric-triangular).

Math: the loss needs only K^2 = exp(-2*d2/(2*sigma^2+EPS)) entries, never K.
With f = 2/(2*sigma^2+EPS):  K2[i,j] = exp(2f*G[i,j]) * exp(-f*r[i]) * exp(-f*r[j]),
G = X @ Y.T.  Every per-class reduction is a quadratic form u^T K2 v, so the
device computes L[c,j] = sum_i u[i,c] * exp(2f*G[i,j] + bias_i)  (fp8 DoubleRow
GEMM -> ScalarE exp with per-partition fp32 bias -> [128,C]x[...] fp8 DoubleRow
reduction matmul into PSUM -> direct DMA out), and the host folds exp(-f*r[j])
plus the right-hand class mask into a tiny dot.  The tt block's uncertainty
weights w_i^2 ride the fp32 bias (+2 ln w_i).

Symmetry: K_ss and K_tt are symmetric, so each unordered 256x256 tile pair is
computed ONCE and counted twice (reduce mask 2.0), self-tiles once (mask 1.0).
Coverage is a wrapped round-robin: rows split into 16 chunks of 256; chunk
r<8 covers the 9-chunk wrapped window [256r, 256r+2304); chunk r>=8 covers 8
chunks [256r, 256r+2048).  Every unordered pair {i,j} of chunks is covered
exactly once.  Core c owns chunks c (pair P0) and 15-c (pair P1); the window
start is rotated away host-side (each core receives column-rotated copies of
the feature matrices), so the device program is identical on all cores (SPMD)
with per-pair window widths 2304/2048.  The st block is not symmetric: both
pairs cover all 4096 columns with mask 1.0 (the 2x in h_mix stays on host).

Per-core outputs L[3, 2, C, 4096] are un-rotated and summed on host.
"""

import numpy as np
import ml_dtypes

import concourse.bacc as bacc
import concourse.tile as tile
import concourse.mybir as mybir
from concourse.bass_utils import run_bass_kernel_spmd

# Problem constants (hardcoded per harness contract)
N = 4096            # N_S == N_T
D = 1024
C = 4
SIGMA = 32.0
EPS = 1e-8
LOG2 = float(np.log(2.0))
LAMBDA_CREDA = 1.0
LAMBDA_ENTROPY = 0.1

NCORES = 8
CH = 256            # row chunk
ROWS = 512          # rows per core (2 chunks)
IT = 128            # i-subtile (PSUM partition dim)
KC = 128            # contraction chunk (PE partition dim)
N_I = 4             # i-subtiles per core
N_K = D // KC       # 8
CP = 16             # class dim padded for DoubleRow stride rule
W0, W1 = 2304, 2048  # symmetric-block window widths for pair 0 / pair 1
TT_LIFT = 32.0      # lifts tt exp values; exactly undone on host

F_SCALE = 2.0 / (2.0 * SIGMA * SIGMA + EPS)   # exponent factor for K^2
ACT_SCALE = float(2.0 * F_SCALE)              # multiplies G inside the exp

BF16 = mybir.dt.bfloat16
FP32 = mybir.dt.float32
FP8 = mybir.dt.float8e4

_COMPILED = {}


def _spans(W):
    """Split window [0, W) into exp-granularity spans of <=1024."""
    out, x = [], 0
    while x < W:
        w = min(1024, W - x)
        out.append((x, w))
        x += w
    return out


# unit = (block, pair, span_start, span_width)
_UNITS = []
for _bk, _Ws in (("ss", (W0, W1)), ("tt", (W0, W1)), ("st", (N, N))):
    for _pr in (0, 1):
        for _x, _w in _spans(_Ws[_pr]):
            _UNITS.append((_bk, _pr, _x, _w))


def _build(repeat=1):
    nc = bacc.Bacc("TRN2", target_bir_lowering=False, debug=False)

    rsA = nc.dram_tensor("rsA", [D, N], FP8, kind="ExternalInput")
    rsB = nc.dram_tensor("rsB", [D, N], FP8, kind="ExternalInput")
    rtA = nc.dram_tensor("rtA", [D, N], FP8, kind="ExternalInput")
    rtB = nc.dram_tensor("rtB", [D, N], FP8, kind="ExternalInput")
    lhs_s = nc.dram_tensor("lhs_s", [D, ROWS], FP8, kind="ExternalInput")
    lhs_t = nc.dram_tensor("lhs_t", [D, ROWS], FP8, kind="ExternalInput")
    lm_s = nc.dram_tensor("lm_s", [IT, N_I, CP], FP8, kind="ExternalInput")
    lm2_s = nc.dram_tensor("lm2_s", [IT, N_I, CP], FP8, kind="ExternalInput")
    lm_t = nc.dram_tensor("lm_t", [IT, N_I, CP], FP8, kind="ExternalInput")
    lm2_t = nc.dram_tensor("lm2_t", [IT, N_I, CP], FP8, kind="ExternalInput")
    bias_s = nc.dram_tensor("bias_s", [IT, N_I], FP32, kind="ExternalInput")
    bias_t = nc.dram_tensor("bias_t", [IT, N_I], FP32, kind="ExternalInput")
    lout = nc.dram_tensor("lout", [3, 2, C, N], FP32, kind="ExternalOutput")
    BKI = {"ss": 0, "tt": 1, "st": 2}

    with tile.TileContext(nc) as tc:
        with (
            tc.tile_pool(name="const", bufs=1) as const,
            tc.tile_pool(name="ep", bufs=6) as epp,
            tc.tile_pool(name="stage", bufs=3) as stp,
            tc.tile_pool(name="gps", bufs=2, space="PSUM") as gps,
            tc.tile_pool(name="lps", bufs=2, space="PSUM") as lps,
        ):
            lhsS = const.tile([KC, N_K, ROWS], FP8)
            nc.sync.dma_start(out=lhsS, in_=lhs_s.ap().rearrange("(k p) i -> p k i", p=KC))
            lhsT = const.tile([KC, N_K, ROWS], FP8)
            nc.sync.dma_start(out=lhsT, in_=lhs_t.ap().rearrange("(k p) i -> p k i", p=KC))
            masks = {}
            for nm, dr in (("lm_s", lm_s), ("lm2_s", lm2_s),
                           ("lm_t", lm_t), ("lm2_t", lm2_t)):
                t = const.tile([IT, N_I, CP], FP8, tag=nm)
                nc.sync.dma_start(out=t, in_=dr.ap())
                masks[nm] = t
            biS = const.tile([IT, N_I], FP32, tag="biS")
            nc.sync.dma_start(out=biS, in_=bias_s.ap())
            biT = const.tile([IT, N_I], FP32, tag="biT")
            nc.sync.dma_start(out=biT, in_=bias_t.ap())

            # rotated rhs copies, SBUF-resident, one tile per 512-col panel
            rhs = {}
            for nm, dr in (("rsA", rsA), ("rsB", rsB), ("rtA", rtA), ("rtB", rtB)):
                ap = dr.ap().rearrange("(k p) j -> p k j", p=KC)
                panels = []
                for j in range(N // 512):
                    t = const.tile([KC, N_K, 512], FP8, tag=f"{nm}{j}")
                    nc.sync.dma_start(out=t, in_=ap[:, :, j * 512:(j + 1) * 512])
                    panels.append(t)
                rhs[nm] = panels

            unit_rhs = {"ss": ("rsA", "rsB"), "tt": ("rtA", "rtB"),
                        "st": ("rtA", "rtB")}
            unit_lhs = {"ss": lhsS, "tt": lhsT, "st": lhsS}
            unit_bias = {"ss": biS, "tt": biT, "st": biS}
            unit_masks = {"ss": ("lm_s", "lm2_s"), "tt": ("lm_t", "lm2_t"),
                          "st": ("lm_s", "lm_s")}

            def emit_gram(u):
                bk, pr, x, w = u
                lhs = unit_lhs[bk]
                rp = rhs[unit_rhs[bk][pr]]
                bi = unit_bias[bk]
                ep2 = epp.tile([IT, 2, 1024], FP8, tag=f"ep_{bk}{pr}", bufs=2,
                               name=f"ep_{bk}{pr}_{x}")
                for h in (0, 1):
                    it = 2 * pr + h
                    gp = gps.tile([IT, 1024], FP32, tag="gp", bufs=2)
                    for px in range(x, x + w, 512):
                        pw = min(512, x + w - px)
                        pan = rp[px // 512]
                        po = px % 512
                        for k2 in range(N_K // 2):
                            nc.tensor.matmul(
                                gp[:, px - x:px - x + pw],
                                lhs[:, 2 * k2:2 * k2 + 2, it * IT:(it + 1) * IT],
                                pan[:, 2 * k2:2 * k2 + 2, po:po + pw],
                                start=(k2 == 0), stop=(k2 == N_K // 2 - 1),
                                perf_mode=mybir.MatmulPerfMode.DoubleRow,
                            )
                    nc.scalar.activation(
                        ep2[:, h, 0:w], gp[:, 0:w],
                        mybir.ActivationFunctionType.Exp,
                        bias=bi[:, it:it + 1], scale=ACT_SCALE,
                    )
                return ep2

            def emit_reduce(u, ep2):
                bk, pr, x, w = u
                m1, m2 = (masks[unit_masks[bk][0]], masks[unit_masks[bk][1]])
                lp = lps.tile([CP, 1024], FP32, tag="lp", bufs=2,
                              name=f"lp_{bk}{pr}_{x}")
                # reduce MMs on the 512 grid; for symmetric blocks the first
                # 256 cols of the window are the self-tile (weight 1), the
                # rest strictly-upper (weight 2)
                mms = []
                for px in range(x, x + w, 512):
                    pw = min(512, x + w - px)
                    if bk == "st":
                        mms.append((px, pw, m1))
                    elif px == 0:
                        mms.append((0, 256, m1))
                        mms.append((256, pw - 256, m2))
                    else:
                        mms.append((px, pw, m2))
                for (cs, cw, m) in mms:
                    nc.tensor.matmul(
                        lp[:, cs - x:cs - x + cw],
                        m[:, 2 * pr:2 * pr + 2, :],
                        ep2[:, :, cs - x:cs - x + cw],
                        start=True, stop=True,
                        perf_mode=mybir.MatmulPerfMode.DoubleRow,
                    )
                st = stp.tile([C, 1024], FP32, tag="st", bufs=3,
                              name=f"st_{bk}{pr}_{x}")
                nc.vector.tensor_copy(st[:, 0:w], lp[0:C, 0:w])
                nc.sync.dma_start(out=lout.ap()[BKI[bk], pr, :, x:x + w],
                                  in_=st[:, 0:w])

            def body():
                prev = None
                for u in _UNITS:
                    ep = emit_gram(u)
                    if prev is not None:
                        emit_reduce(*prev)
                    prev = (u, ep)
                emit_reduce(*prev)

            if repeat == 1:
                body()
            else:
                with tc.For_i(0, repeat, 1):
                    body()

    nc.compile()
    return nc


def _get_nc(repeat=1):
    if repeat not in _COMPILED:
        _COMPILED[repeat] = _build(repeat)
    return _COMPILED[repeat]


def _host_prep(features_s, logits_s, features_t, logits_t, labels_s):
    fs = np.asarray(features_s, dtype=np.float32)
    ft = np.asarray(features_t, dtype=np.float32)
    lt = np.asarray(logits_t, dtype=np.float32)
    lab = np.asarray(labels_s).astype(np.int64)

    # target softmax / pseudo labels / uncertainty weights (host, fp64)
    z = lt.astype(np.float64)
    z = z - z.max(axis=1, keepdims=True)
    pt = np.exp(z)
    pt /= pt.sum(axis=1, keepdims=True)
    pseudo = np.argmax(pt, axis=1)
    h2p = -np.log(np.sum(pt * pt, axis=1) + EPS) / LOG2
    h2max = np.log(float(C)) / LOG2
    w = 1.0 - h2p / (h2max + EPS)

    ms = np.zeros((N, C), dtype=np.float64)
    ms[np.arange(N), lab] = 1.0
    mt = np.zeros((N, C), dtype=np.float64)
    mt[np.arange(N), pseudo] = 1.0
    wt2 = mt * (w * w)[:, None]          # [N, C] == Wt2.T of the reference

    rs = np.sum(fs.astype(np.float64) ** 2, axis=1)
    rt = np.sum(ft.astype(np.float64) ** 2, axis=1)
    es = np.exp(-F_SCALE * rs)
    et = np.exp(-F_SCALE * rt)

    # tt-block row weights w^2 (and a xTT_LIFT lift) ride the fp32 ACT bias:
    # exp(2fG - f*rt_i + 2 ln w_i + ln TT_LIFT)
    logw2 = np.where(w > 0, 2.0 * np.log(np.maximum(w, 1e-300)), -1e30)
    bias_tt = -F_SCALE * rt + logw2 + np.log(TT_LIFT)

    xsT = np.ascontiguousarray(fs.T).astype(ml_dtypes.float8_e4m3)
    xtT = np.ascontiguousarray(ft.T).astype(ml_dtypes.float8_e4m3)

    msp = np.zeros((N, CP)); msp[:, :C] = ms
    mtp = np.zeros((N, CP)); mtp[:, :C] = mt

    jidx = np.arange(N)
    in_maps = []
    for c in range(NCORES):
        ca, cb = c, 15 - c
        rowsel = np.r_[CH * ca:CH * ca + CH, CH * cb:CH * cb + CH]
        rotA, rotB = CH * ca, CH * cb

        def msk(mp):
            return np.ascontiguousarray(
                mp[rowsel].reshape(N_I, IT, CP).transpose(1, 0, 2)
            ).astype(ml_dtypes.float8_e4m3)

        in_maps.append({
            "rsA": np.ascontiguousarray(xsT[:, (jidx + rotA) % N]),
            "rsB": np.ascontiguousarray(xsT[:, (jidx + rotB) % N]),
            "rtA": np.ascontiguousarray(xtT[:, (jidx + rotA) % N]),
            "rtB": np.ascontiguousarray(xtT[:, (jidx + rotB) % N]),
            "lhs_s": np.ascontiguousarray(xsT[:, rowsel]),
            "lhs_t": np.ascontiguousarray(xtT[:, rowsel]),
            "lm_s": msk(msp),
            "lm2_s": msk(2.0 * msp),
            "lm_t": msk(mtp),
            "lm2_t": msk(2.0 * mtp),
            "bias_s": np.ascontiguousarray(
                (-F_SCALE * rs[rowsel]).reshape(N_I, IT).T).astype(np.float32),
            "bias_t": np.ascontiguousarray(
                bias_tt[rowsel].reshape(N_I, IT).T).astype(np.float32),
        })

    S = 512
    xs8 = xsT[:, :S].astype(np.float64).T      # [S, D] dequantized fp8
    xt8 = xtT[:, :S].astype(np.float64).T
    f8 = lambda a: a.astype(ml_dtypes.float8_e4m3).astype(np.float64)

    def _calib(Xrow, Ycol, brow):
        E = np.exp(ACT_SCALE * (Xrow @ Ycol.T) + brow[:, None])
        return float(f8(E).sum() / E.sum())

    cal = np.array([
        _calib(xs8, xs8, -F_SCALE * rs[:S]),
        _calib(xt8, xt8, bias_tt[:S]),
        _calib(xs8, xt8, -F_SCALE * rs[:S]),
    ])

    aux = dict(ms=ms, mt=mt, wt2=wt2, es=es, et=et, w=w, lab=lab, pt=pt,
               cal=cal)
    return in_maps, aux


def _gather_L(results):
    """Un-rotate per-core pair outputs and sum into L[3, C, N]."""
    L = np.zeros((3, C, N), dtype=np.float64)
    for c, r in enumerate(results):
        lo = r["lout"].astype(np.float64)      # [3, 2, C, N]
        rotA, rotB = CH * c, CH * (15 - c)
        for bk in range(3):
            L[bk] += np.roll(lo[bk, 0], rotA, axis=-1)
            L[bk] += np.roll(lo[bk, 1], rotB, axis=-1)
    return L


def _host_finish(L, aux, logits_s):
    ms, mt, wt2 = aux["ms"], aux["mt"], aux["wt2"]
    es, et, lab, pt = aux["es"], aux["et"], aux["lab"], aux["pt"]

    # right-hand side masks with the column exp factor folded in
    rm_ss = ms * es[:, None]             # [N, C]
    rm_tt = wt2 * et[:, None]
    rm_st = mt * et[:, None]

    cal = aux["cal"]
    ss_s = np.einsum("cj,jc->c", L[0], rm_ss) / cal[0]
    ss_t = np.einsum("cj,jc->c", L[1], rm_tt) / TT_LIFT / cal[1]
    ss_st = np.einsum("cj,jc->c", L[2], rm_st) / cal[2]

    n_s = ms.sum(axis=0)
    n_t = mt.sum(axis=0)
    tr_s = n_s
    tr_t = wt2.sum(axis=0)

    def h2(tr, sumsq):
        info = sumsq / (tr + EPS) ** 2
        return -np.log(info + EPS) / LOG2

    h_s = h2(tr_s, ss_s)
    h_t = h2(tr_t, ss_t)
    h_mix = h2(tr_s + tr_t, ss_s + 2.0 * ss_st + ss_t)
    per_class = h_mix - 0.5 * (h_s + h_t)
    valid = (n_s >= 2) & (n_t >= 2)
    n_valid = float(valid.sum())
    creda_sum = float(np.where(valid, per_class, 0.0).sum())
    loss_creda = creda_sum / max(n_valid, 1.0) if n_valid > 0 else 0.0

    # source cross entropy (host, fp64)
    zs = np.asarray(logits_s, dtype=np.float64)
    zs = zs - zs.max(axis=1, keepdims=True)
    lse = np.log(np.exp(zs).sum(axis=1))
    logp = zs - lse[:, None]
    loss_cls = -float(np.mean(logp[np.arange(N), lab]))

    # target entropy
    loss_ent = -float(np.mean(np.sum(pt * np.log(pt + EPS), axis=1)))

    total = loss_cls + LAMBDA_CREDA * loss_creda + LAMBDA_ENTROPY * loss_ent
    return np.array(total, dtype=np.float32)


def run(inputs, trace=False, repeat=1):
    """Full pipeline; returns (loss, BassKernelResults)."""
    in_maps, aux = _host_prep(**inputs)
    nc = _get_nc(repeat)
    res = run_bass_kernel_spmd(
        nc, in_maps, core_ids=list(range(NCORES)), trace=trace,
    )
    L = _gather_L(res.results)
    loss = _host_finish(L, aux, inputs["logits_s"])
    return loss, res


def kernel(**inputs) -> np.ndarray:
    loss, _ = run(inputs, trace=False)
    return loss


# revision 15
# speedup vs baseline: 3.3092x; 1.8916x over previous
"""CREDA loss kernel for Trainium2 (8 NeuronCores, SPMD, class-blocked).

Math: the loss needs only K^2 = exp(-2*d2/(2*sigma^2+EPS)) entries, never K.
With f = 2/(2*sigma^2+EPS):  K2[i,j] = exp(2f*G[i,j]) * exp(-f*r[i]) * exp(-f*r[j]),
G = X @ Y.T.  Every per-class reduction is a quadratic form u^T K2 v computed
as  sum_j v[j] * (sum_i u[i] * exp(2f*G[i,j] + bias_i)):  fp8 DoubleRow GEMM
-> ScalarE exp with per-partition fp32 bias -> fp8 reduction matmul -> host dot.
The tt block's uncertainty weights w_i^2 ride the fp32 bias (+2 ln w_i).

Class blocking: the class masks are one-hots, so only SAME-class (i,j) pairs
contribute.  Rows/cols are sorted by class (labels for fs, pseudo-labels for
ft), padded per class to nch_k*128 rows.  Each NxN kernel block splits into
per-class blocks (~4x less work).  K_ss/K_tt are symmetric: within a class
block a wrapped round-robin covers every unordered 128-row-chunk pair once
(chunk r covers a contiguous wrapped window of chunks; self-chunk counted
once, cross chunks doubled via a 2.0 reduce mask).  K_st is not symmetric:
full per-class blocks, weight 1 (the 2x in h_mix stays on host).

The resulting strips are chopped into [128 rows x W cols] PIECES with
W in {512, 384, 256, 128} and distributed round-robin over the 8 cores,
padded with zero-mask dummy pieces so every core gets the identical piece
sequence (SPMD).  Block type / class / chunk / rotation / doubling live
entirely in host-packed data (per-piece lhs rows, moving cols, two reduce
masks, bias, and host-side right weights).  Device program per piece:
Gram [128 x W] (K=1024, fp8 DoubleRow) -> exp -> reduce matmuls
([0:128) mask1, [128:W) mask2) -> DVE copy -> DMA to lout[piece].
"""

import numpy as np
import ml_dtypes

import concourse.bacc as bacc
import concourse.tile as tile
import concourse.mybir as mybir
from concourse.bass_utils import run_bass_kernel_spmd

# Problem constants (hardcoded per harness contract)
N = 4096            # N_S == N_T
D = 1024
C = 4
SIGMA = 32.0
EPS = 1e-8
LOG2 = float(np.log(2.0))
LAMBDA_CREDA = 1.0
LAMBDA_ENTROPY = 0.1

NCORES = 8
IT = 128            # rows per piece (PSUM partition dim)
KC = 128            # contraction chunk (PE partition dim)
N_K = D // KC       # 8
CP = 16             # class dim padded (stationary free of reduce matmuls)
TT_LIFT = 32.0      # lifts tt exp values; exactly undone on host
WIDTHS = (512, 384, 256, 128)

F_SCALE = 2.0 / (2.0 * SIGMA * SIGMA + EPS)   # exponent factor for K^2
ACT_SCALE = float(2.0 * F_SCALE)              # multiplies G inside the exp

BF16 = mybir.dt.bfloat16
FP32 = mybir.dt.float32
FP8 = mybir.dt.float8e4

_COMPILED = {}
_LAST_GEOM = None    # set by _host_prep; used by _get_nc for the bench path


def _build(geom, repeat=1, gp_bufs=3, lp_bufs=3, st_bufs=4, pipe=1):
    """geom: per-core piece counts per width, e.g. (16, 3, 2, 5)."""
    widths = []
    for wd, n in zip(WIDTHS, geom):
        widths += [wd] * n
    np_ = len(widths)
    mv_tot = sum(widths)
    offs = np.concatenate([[0], np.cumsum(widths)]).astype(int)

    nc = bacc.Bacc("TRN2", target_bir_lowering=False, debug=False)
    mov = nc.dram_tensor("mov", [D, mv_tot], FP8, kind="ExternalInput")
    lhsA = nc.dram_tensor("lhsA", [D, np_ * IT], FP8, kind="ExternalInput")
    lm1 = nc.dram_tensor("lm1", [IT, np_, CP], FP8, kind="ExternalInput")
    lm2 = nc.dram_tensor("lm2", [IT, np_, CP], FP8, kind="ExternalInput")
    biA = nc.dram_tensor("biA", [IT, np_], FP32, kind="ExternalInput")
    lout = nc.dram_tensor("lout", [np_, C, 512], FP32, kind="ExternalOutput")

    with tile.TileContext(nc) as tc:
        with (
            tc.tile_pool(name="const", bufs=1) as const,
            tc.tile_pool(name="ep", bufs=6) as epp,
            tc.tile_pool(name="stage", bufs=st_bufs) as stp,
            tc.tile_pool(name="gps", bufs=2, space="PSUM") as gps,
            tc.tile_pool(name="lps", bufs=2, space="PSUM") as lps,
        ):
            lhsT = const.tile([KC, N_K, np_ * IT], FP8, tag="lhs")
            nc.sync.dma_start(out=lhsT, in_=lhsA.ap().rearrange("(k p) i -> p k i", p=KC))
            mvT = const.tile([KC, N_K, mv_tot], FP8, tag="mov")
            nc.sync.dma_start(out=mvT, in_=mov.ap().rearrange("(k p) j -> p k j", p=KC))
            lm1T = const.tile([IT, np_, CP], FP8, tag="lm1")
            nc.sync.dma_start(out=lm1T, in_=lm1.ap())
            lm2T = const.tile([IT, np_, CP], FP8, tag="lm2")
            nc.sync.dma_start(out=lm2T, in_=lm2.ap())
            biT = const.tile([IT, np_], FP32, tag="biA")
            nc.sync.dma_start(out=biT, in_=biA.ap())

            def emit_gram(p):
                w, off = widths[p], offs[p]
                ep = epp.tile([IT, 512], FP8, tag=f"ep{p % 3}", bufs=2,
                              name=f"ep_{p}")
                gp = gps.tile([IT, 512], FP32, tag="gp", bufs=gp_bufs)
                for k2 in range(N_K // 2):
                    nc.tensor.matmul(
                        gp[:, 0:w],
                        lhsT[:, 2 * k2:2 * k2 + 2, p * IT:(p + 1) * IT],
                        mvT[:, 2 * k2:2 * k2 + 2, off:off + w],
                        start=(k2 == 0), stop=(k2 == N_K // 2 - 1),
                        perf_mode=mybir.MatmulPerfMode.DoubleRow,
                    )
                nc.scalar.activation(
                    ep[:, 0:w], gp[:, 0:w],
                    mybir.ActivationFunctionType.Exp,
                    bias=biT[:, p:p + 1], scale=ACT_SCALE,
                )
                return ep

            def emit_reduce(p, ep):
                w = widths[p]
                lp = lps.tile([CP, 512], FP32, tag="lp", bufs=lp_bufs,
                              name=f"lp_{p}")
                nc.tensor.matmul(lp[:, 0:IT], lm1T[:, p, :], ep[:, 0:IT],
                                 start=True, stop=True)
                if w > IT:
                    nc.tensor.matmul(lp[:, IT:w], lm2T[:, p, :], ep[:, IT:w],
                                     start=True, stop=True)
                st = stp.tile([C, 512], FP32, tag="st", bufs=st_bufs,
                              name=f"st_{p}")
                nc.vector.tensor_copy(st[:, 0:w], lp[0:C, 0:w])
                nc.sync.dma_start(out=lout.ap()[p, :, 0:w], in_=st[:, 0:w])

            def body():
                pend = []
                for p in range(np_):
                    ep = emit_gram(p)
                    pend.append((p, ep))
                    if len(pend) > pipe:
                        emit_reduce(*pend.pop(0))
                for pe_ in pend:
                    emit_reduce(*pe_)

            if repeat == 1:
                body()
            else:
                with tc.For_i(0, repeat, 1):
                    body()

    nc.compile()
    return nc


def _get_nc(repeat=1, geom=None):
    if geom is None:
        geom = _LAST_GEOM
    key = (tuple(geom), repeat)
    if key not in _COMPILED:
        _COMPILED[key] = _build(geom, repeat=repeat)
    return _COMPILED[key]


def _class_index(classes):
    """idx[k] = padded row-index array (len nch_k*128, -1 = pad)."""
    order = np.argsort(classes, kind="stable")
    out = []
    for k in range(C):
        rows = order[classes[order] == k]
        nch = max(1, (len(rows) + IT - 1) // IT)
        idx = np.full(nch * IT, -1, dtype=np.int64)
        idx[:len(rows)] = rows
        out.append(idx)
    return out


def _sym_strips(idx):
    """Wrapped round-robin strips covering each unordered chunk pair once.

    Returns list of (rows, cols) index arrays; cols[0:128] is the self-chunk
    (weight 1), the rest weight 2.
    """
    nch = len(idx) // IT
    M = nch * IT
    strips = []
    for r in range(nch):
        if nch % 2 == 1:
            units = (nch + 1) // 2
        else:
            units = nch // 2 + 1 if r < nch // 2 else nch // 2
        cols = idx[(r * IT + np.arange(units * IT)) % M]
        strips.append((idx[r * IT:(r + 1) * IT], cols))
    return strips


def _host_prep(features_s, logits_s, features_t, logits_t, labels_s):
    fs = np.asarray(features_s, dtype=np.float32)
    ft = np.asarray(features_t, dtype=np.float32)
    lt = np.asarray(logits_t, dtype=np.float32)
    lab = np.asarray(labels_s).astype(np.int64)

    # target softmax / pseudo labels / uncertainty weights (host, fp64)
    z = lt.astype(np.float64)
    z = z - z.max(axis=1, keepdims=True)
    pt = np.exp(z)
    pt /= pt.sum(axis=1, keepdims=True)
    pseudo = np.argmax(pt, axis=1)
    h2p = -np.log(np.sum(pt * pt, axis=1) + EPS) / LOG2
    h2max = np.log(float(C)) / LOG2
    w = 1.0 - h2p / (h2max + EPS)

    ms = np.zeros((N, C), dtype=np.float64)
    ms[np.arange(N), lab] = 1.0
    mt = np.zeros((N, C), dtype=np.float64)
    mt[np.arange(N), pseudo] = 1.0
    wt2 = mt * (w * w)[:, None]

    rs = np.sum(fs.astype(np.float64) ** 2, axis=1)
    rt = np.sum(ft.astype(np.float64) ** 2, axis=1)
    es = np.exp(-F_SCALE * rs)
    et = np.exp(-F_SCALE * rt)

    logw2 = np.where(w > 0, 2.0 * np.log(np.maximum(w, 1e-300)), -1e30)
    bias_tt = -F_SCALE * rt + logw2 + np.log(TT_LIFT)
    bias_ss = -F_SCALE * rs

    xsT = np.ascontiguousarray(fs.T).astype(ml_dtypes.float8_e4m3)
    xtT = np.ascontiguousarray(ft.T).astype(ml_dtypes.float8_e4m3)

    idx_s = _class_index(lab)
    idx_t = _class_index(pseudo)

    # ---- build the global piece list -------------------------------------
    # piece = dict(w, bk, k, rows[128], cols[w], first)
    pieces = []

    def chop(bk, k, rows, cols, sym_first):
        x = 0
        while x < len(cols):
            for wd in WIDTHS:
                if x + wd <= len(cols):
                    break
            pieces.append(dict(
                w=wd, bk=bk, k=k, rows=rows, cols=cols[x:x + wd],
                first=(sym_first and x == 0),
            ))
            x += wd

    for bk, idx in (("ss", idx_s), ("tt", idx_t)):
        for k in range(C):
            for rows, cols in _sym_strips(idx[k]):
                chop(bk, k, rows, cols, True)
    for k in range(C):
        nch_s = len(idx_s[k]) // IT
        for r in range(nch_s):
            rows = idx_s[k][r * IT:(r + 1) * IT]
            chop("st", k, rows, idx_t[k], False)

    # ---- distribute per width type round-robin over cores, pad dummies ---
    per_core = [[] for _ in range(NCORES)]
    geom = []
    for wd in WIDTHS:
        ps = [p for p in pieces if p["w"] == wd]
        n = (len(ps) + NCORES - 1) // NCORES
        geom.append(n)
        dummy = dict(w=wd, bk="ss", k=0, rows=np.full(IT, -1),
                     cols=np.full(wd, -1), first=False, dummy=True)
        while len(ps) < n * NCORES:
            ps.append(dict(dummy))
        for i, p in enumerate(ps):
            per_core[i % NCORES].append(p)
    geom = tuple(geom)
    np_ = sum(geom)
    mv_tot = 0
    widths = []
    for wd, n in zip(WIDTHS, geom):
        widths += [wd] * n
        mv_tot += wd * n
    # per_core lists are width-sorted because pieces were appended per width

    def feat(xT, cols):
        out = np.zeros((D, len(cols)), dtype=ml_dtypes.float8_e4m3)
        real = cols >= 0
        out[:, real] = xT[:, cols[real]]
        return out

    in_maps = []
    piece_meta = []
    for c in range(NCORES):
        mov = np.zeros((D, mv_tot), dtype=ml_dtypes.float8_e4m3)
        lhsA = np.zeros((D, np_ * IT), dtype=ml_dtypes.float8_e4m3)
        lm1 = np.zeros((IT, np_, CP), dtype=np.float64)
        lm2 = np.zeros((IT, np_, CP), dtype=np.float64)
        biA = np.zeros((IT, np_), dtype=np.float32)
        meta = []
        off = 0
        for p_i, p in enumerate(per_core[c]):
            wd, bk, k = p["w"], p["bk"], p["k"]
            rows, cols = p["rows"], p["cols"]
            if p.get("dummy"):
                meta.append((bk, k, np.zeros(wd)))
                off += wd
                continue
            rowT = xsT if bk in ("ss", "st") else xtT
            colT = xsT if bk == "ss" else xtT
            mov[:, off:off + wd] = feat(colT, cols)
            lhsA[:, p_i * IT:(p_i + 1) * IT] = feat(rowT, rows)
            real = rows >= 0
            rr = rows[real]
            lm1[real, p_i, k] = 1.0
            lm2[real, p_i, k] = 1.0 if bk == "st" else 2.0
            if bk != "st" and not p["first"]:
                lm1[real, p_i, k] = lm2[real, p_i, k]
            biA[real, p_i] = (bias_tt if bk == "tt" else bias_ss)[rr]
            vfull = es if bk == "ss" else ((w * w) * et if bk == "tt" else et)
            vcol = np.zeros(wd)
            realc = cols >= 0
            vcol[realc] = vfull[cols[realc]]
            meta.append((bk, k, vcol))
            off += wd
        in_maps.append({
            "mov": mov, "lhsA": lhsA,
            "lm1": np.ascontiguousarray(lm1).astype(ml_dtypes.float8_e4m3),
            "lm2": np.ascontiguousarray(lm2).astype(ml_dtypes.float8_e4m3),
            "biA": biA,
        })
        piece_meta.append(meta)

    S = 512
    xs8 = xsT[:, :S].astype(np.float64).T
    xt8 = xtT[:, :S].astype(np.float64).T
    f8 = lambda a: a.astype(ml_dtypes.float8_e4m3).astype(np.float64)

    def _calib(Xrow, Ycol, brow):
        E = np.exp(ACT_SCALE * (Xrow @ Ycol.T) + brow[:, None])
        return float(f8(E).sum() / E.sum())

    cal = {
        "ss": _calib(xs8, xs8, bias_ss[:S]),
        "tt": _calib(xt8, xt8, bias_tt[:S]),
        "st": _calib(xs8, xt8, bias_ss[:S]),
    }

    global _LAST_GEOM
    _LAST_GEOM = geom
    aux = dict(ms=ms, mt=mt, wt2=wt2, lab=lab, pt=pt, cal=cal,
               piece_meta=piece_meta, geom=geom)
    return in_maps, aux


def _host_finish(results, aux, logits_s):
    ms, mt, wt2 = aux["ms"], aux["mt"], aux["wt2"]
    lab, pt, cal = aux["lab"], aux["pt"], aux["cal"]

    acc = {"ss": np.zeros(C), "tt": np.zeros(C), "st": np.zeros(C)}
    for c, r in enumerate(results):
        L = r["lout"].astype(np.float64)       # [np, C, 512]
        for p_i, (bk, k, vcol) in enumerate(aux["piece_meta"][c]):
            acc[bk][k] += L[p_i, k, :len(vcol)] @ vcol

    ss_s = acc["ss"] / cal["ss"]
    ss_t = acc["tt"] / TT_LIFT / cal["tt"]
    ss_st = acc["st"] / cal["st"]

    n_s = ms.sum(axis=0)
    n_t = mt.sum(axis=0)
    tr_s = n_s
    tr_t = wt2.sum(axis=0)

    def h2(tr, sumsq):
        info = sumsq / (tr + EPS) ** 2
        return -np.log(info + EPS) / LOG2

    h_s = h2(tr_s, ss_s)
    h_t = h2(tr_t, ss_t)
    h_mix = h2(tr_s + tr_t, ss_s + 2.0 * ss_st + ss_t)
    per_class = h_mix - 0.5 * (h_s + h_t)
    valid = (n_s >= 2) & (n_t >= 2)
    n_valid = float(valid.sum())
    creda_sum = float(np.where(valid, per_class, 0.0).sum())
    loss_creda = creda_sum / max(n_valid, 1.0) if n_valid > 0 else 0.0

    zs = np.asarray(logits_s, dtype=np.float64)
    zs = zs - zs.max(axis=1, keepdims=True)
    lse = np.log(np.exp(zs).sum(axis=1))
    logp = zs - lse[:, None]
    loss_cls = -float(np.mean(logp[np.arange(N), lab]))

    loss_ent = -float(np.mean(np.sum(pt * np.log(pt + EPS), axis=1)))

    total = loss_cls + LAMBDA_CREDA * loss_creda + LAMBDA_ENTROPY * loss_ent
    return np.array(total, dtype=np.float32)


def run(inputs, trace=False, repeat=1):
    """Full pipeline; returns (loss, BassKernelResults)."""
    in_maps, aux = _host_prep(**inputs)
    nc = _get_nc(repeat, geom=aux["geom"])
    res = run_bass_kernel_spmd(
        nc, in_maps, core_ids=list(range(NCORES)), trace=trace,
    )
    loss = _host_finish(res.results, aux, inputs["logits_s"])
    return loss, res


def kernel(**inputs) -> np.ndarray:
    loss, _ = run(inputs, trace=False)
    return loss


# revision 23
# speedup vs baseline: 5.6034x; 1.6933x over previous
"""CREDA loss kernel for Trainium2 (8 NeuronCores, SPMD, class-blocked).

Math: the loss needs only K^2 entries: with f = 2/(2*sigma^2+EPS),
K2[i,j] = exp(2f*G[i,j]) * exp(-f*r[i]) * exp(-f*r[j]), G = X @ Y.T, and
every per-class reduction is a quadratic form u^T K2 v.  The device computes,
for a tile of columns j and a window of rows i,
    L[j] = sum_i exp(2f*G[j,i]) * mw[i],
with mw[i] = u[i] * wt[i] * exp(-f*r[i]) (class mask, symmetry doubling
weight, and row norm factor folded into one bf16 vector).  The host applies
the column factors v[j] (mask * exp(-f*r[j])).

Engine mapping: Gram on PE (fp8 DoubleRow, K=1024, columns are the PSUM
partition dim) -> exp on ScalarE (scale=2f, bf16 output) -> weighted row-sum on DVE
(scalar_tensor_tensor accum, the ONLY consumer) -> one [128, npieces] DMA.
The PE runs nothing but identical DoubleRow matmuls - no mode switches, no
PSUM reduce tiles.

Class blocking: only SAME-class (i,j) pairs contribute (the masks are
one-hot).  Rows/cols are sorted by class (labels for fs, pseudo-labels for
ft) and padded per class to nch_k*128.  K_ss/K_tt are symmetric: a wrapped
round-robin covers every unordered 128-chunk pair once (cross-chunk rows get
doubling weight 2, the self-chunk weight 1, inside mw).  K_st is full per
class with weight 1 (the 2x in h_mix stays on host).

A device piece = [128 class-block columns] x [their full row window].
Pieces are sorted by window width, grouped into rounds of 8 (one piece per
core), each round padded to a common width, so all cores run the identical
program (SPMD); block/class/chunk identity lives in host-packed data.
Padded rows carry mw=0; padded/dummy columns get host weight 0.
"""

import numpy as np
import ml_dtypes

import concourse.bacc as bacc
import concourse.tile as tile
import concourse.mybir as mybir
from concourse.bass_utils import run_bass_kernel_spmd

# Problem constants (hardcoded per harness contract)
N = 4096            # N_S == N_T
D = 1024
C = 4
SIGMA = 32.0
EPS = 1e-8
LOG2 = float(np.log(2.0))
LAMBDA_CREDA = 1.0
LAMBDA_ENTROPY = 0.1

NCORES = 8
IT = 128            # columns per piece (PSUM partition dim)
KC = 128            # contraction chunk (PE partition dim)
N_K = D // KC       # 8
WCAP = 1536          # max row-window per piece (PSUM: [128, WCAP] fp32 = 3 banks)

F_SCALE = 2.0 / (2.0 * SIGMA * SIGMA + EPS)
ACT_SCALE = float(2.0 * F_SCALE)

BF16 = mybir.dt.bfloat16
FP32 = mybir.dt.float32
FP8 = mybir.dt.float8e4

_COMPILED = {}
_LAST_GEOM = None


def _build(geom, repeat=1, gp_bufs=2, st_bufs=2, pipe=1):
    """geom: tuple of round widths (one piece of that width per core/round)."""
    widths = list(geom)
    np_ = len(widths)
    wmax = max(widths)
    mv_tot = sum(widths)
    offs = np.concatenate([[0], np.cumsum(widths)]).astype(int)

    nc = bacc.Bacc("TRN2", target_bir_lowering=False, debug=False)
    mov = nc.dram_tensor("mov", [D, mv_tot], FP8, kind="ExternalInput")
    lhsA = nc.dram_tensor("lhsA", [D, np_ * IT], FP8, kind="ExternalInput")
    mw = nc.dram_tensor("mw", [IT, mv_tot], BF16, kind="ExternalInput")
    lout = nc.dram_tensor("lout", [IT, np_], FP32, kind="ExternalOutput")

    with tile.TileContext(nc) as tc:
        with (
            tc.tile_pool(name="const", bufs=1) as const,
            tc.tile_pool(name="ep", bufs=6) as epp,
            tc.tile_pool(name="stage", bufs=st_bufs) as stp,
            tc.tile_pool(name="gps", bufs=2, space="PSUM") as gps,
        ):
            lhsT = const.tile([KC, N_K, np_ * IT], FP8, tag="lhs")
            nc.sync.dma_start(out=lhsT, in_=lhsA.ap().rearrange("(k p) i -> p k i", p=KC))
            mvT = const.tile([KC, N_K, mv_tot], FP8, tag="mov")
            nc.sync.dma_start(out=mvT, in_=mov.ap().rearrange("(k p) j -> p k j", p=KC))
            mwT = const.tile([IT, mv_tot], BF16, tag="mw")
            nc.sync.dma_start(out=mwT, in_=mw.ap())

            def body():
                stage = stp.tile([IT, np_], FP32, tag="stage", bufs=st_bufs,
                                 name="stage")
                for p in range(np_):
                    w, off = widths[p], offs[p]
                    ep = epp.tile([IT, wmax], BF16, tag=f"ep{p % 3}", bufs=2,
                                  name=f"ep_{p}")
                    gp = gps.tile([IT, wmax], FP32, tag="gp", bufs=gp_bufs)
                    for a in range(0, w, 512):
                        b = min(a + 512, w)
                        for k2 in range(N_K // 2):
                            nc.tensor.matmul(
                                gp[:, a:b],
                                lhsT[:, 2 * k2:2 * k2 + 2, p * IT:(p + 1) * IT],
                                mvT[:, 2 * k2:2 * k2 + 2, off + a:off + b],
                                start=(k2 == 0), stop=(k2 == N_K // 2 - 1),
                                perf_mode=mybir.MatmulPerfMode.DoubleRow,
                            )
                    nc.scalar.activation(
                        ep[:, 0:w], gp[:, 0:w],
                        mybir.ActivationFunctionType.Exp,
                        scale=ACT_SCALE,
                    )
                    sc = stp.tile([IT, wmax], BF16, tag=f"sc{p % 2}", bufs=2,
                                  name=f"sc_{p}")
                    nc.vector.scalar_tensor_tensor(
                        out=sc[:, 0:w],
                        in0=ep[:, 0:w],
                        scalar=1.0,
                        in1=mwT[:, off:off + w],
                        op0=mybir.AluOpType.mult,
                        op1=mybir.AluOpType.mult,
                        accum_out=stage[:, p:p + 1],
                    )
                nc.sync.dma_start(out=lout.ap(), in_=stage)

            if repeat == 1:
                body()
            else:
                with tc.For_i(0, repeat, 1):
                    body()

    nc.compile()
    return nc


def _get_nc(repeat=1, geom=None):
    if geom is None:
        geom = _LAST_GEOM
    key = (tuple(geom), repeat)
    if key not in _COMPILED:
        _COMPILED[key] = _build(geom, repeat=repeat)
    return _COMPILED[key]


def _class_index(classes):
    """idx[k] = padded row-index array (len nch_k*128, -1 = pad)."""
    order = np.argsort(classes, kind="stable")
    out = []
    for k in range(C):
        rows = order[classes[order] == k]
        nch = max(1, (len(rows) + IT - 1) // IT)
        idx = np.full(nch * IT, -1, dtype=np.int64)
        idx[:len(rows)] = rows
        out.append(idx)
    return out


def _row_windows(nch):
    """For each col chunk j: list of (row chunk r, doubling weight).

    Chunk r covers col chunks r..r+u_r-1 (wrapped); the transposed view
    gives, per column chunk j, the set of covering row chunks.  Weight 2
    for cross chunks (computed once, counted twice), 1 for the self chunk.
    """
    wins = [[] for _ in range(nch)]
    for r in range(nch):
        if nch % 2 == 1:
            u = (nch + 1) // 2
        else:
            u = nch // 2 + 1 if r < nch // 2 else nch // 2
        for d in range(u):
            j = (r + d) % nch
            wins[j].append((r, 1.0 if d == 0 else 2.0))
    return wins


def _host_prep(features_s, logits_s, features_t, logits_t, labels_s):
    fs = np.asarray(features_s, dtype=np.float32)
    ft = np.asarray(features_t, dtype=np.float32)
    lt = np.asarray(logits_t, dtype=np.float32)
    lab = np.asarray(labels_s).astype(np.int64)

    z = lt.astype(np.float64)
    z = z - z.max(axis=1, keepdims=True)
    pt = np.exp(z)
    pt /= pt.sum(axis=1, keepdims=True)
    pseudo = np.argmax(pt, axis=1)
    h2p = -np.log(np.sum(pt * pt, axis=1) + EPS) / LOG2
    h2max = np.log(float(C)) / LOG2
    w = 1.0 - h2p / (h2max + EPS)

    ms = np.zeros((N, C), dtype=np.float64)
    ms[np.arange(N), lab] = 1.0
    mt = np.zeros((N, C), dtype=np.float64)
    mt[np.arange(N), pseudo] = 1.0
    wt2 = mt * (w * w)[:, None]

    rs = np.sum(fs.astype(np.float64) ** 2, axis=1)
    rt = np.sum(ft.astype(np.float64) ** 2, axis=1)
    es = np.exp(-F_SCALE * rs)
    et = np.exp(-F_SCALE * rt)

    xsT = np.ascontiguousarray(fs.T).astype(ml_dtypes.float8_e4m3)
    xtT = np.ascontiguousarray(ft.T).astype(ml_dtypes.float8_e4m3)

    idx_s = _class_index(lab)
    idx_t = _class_index(pseudo)

    # row-weight vectors in global row order (mask is implicit: rows of the
    # right class only enter via the class-sorted index arrays)
    mw_ss = es.copy()                    # u=1 on real rows
    mw_tt = (w * w) * et
    mw_st = es.copy()

    # ---- build transposed pieces: 128 cols x row window ------------------
    # piece = (W, cols[128], rowidx[W], roww[W], bk, k)
    pieces = []

    def add_piece(bk, k, cols, rowidx, roww):
        pieces.append(dict(W=len(rowidx), bk=bk, k=k, cols=cols,
                           rows=np.asarray(rowidx), roww=np.asarray(roww)))

    for bk, idx in (("ss", idx_s), ("tt", idx_t)):
        for k in range(C):
            nch = len(idx[k]) // IT
            wins = _row_windows(nch)
            for j in range(nch):
                cols = idx[k][j * IT:(j + 1) * IT]
                rowidx, roww = [], []
                for (r, wt) in wins[j]:
                    rowidx.append(idx[k][r * IT:(r + 1) * IT])
                    roww.append(np.full(IT, wt))
                add_piece(bk, k, cols, np.concatenate(rowidx),
                          np.concatenate(roww))
    for k in range(C):
        nch_t = len(idx_t[k]) // IT
        for j in range(nch_t):
            cols = idx_t[k][j * IT:(j + 1) * IT]
            add_piece("st", k, cols, idx_s[k], np.ones(len(idx_s[k])))

    # ---- split row windows larger than WCAP (partial sums add on host) ---
    split = []
    for p in pieces:
        if p["W"] <= WCAP:
            split.append(p)
            continue
        for a in range(0, p["W"], WCAP):
            b = min(a + WCAP, p["W"])
            split.append(dict(W=b - a, bk=p["bk"], k=p["k"], cols=p["cols"],
                              rows=p["rows"][a:b], roww=p["roww"][a:b]))
    pieces = split

    # ---- rounds of 8: sort desc by W, pad round members to round max -----
    pieces.sort(key=lambda p: -p["W"])
    while len(pieces) % NCORES:
        pieces.append(dict(W=pieces[-1]["W"], bk="ss", k=0,
                           cols=np.full(IT, -1),
                           rows=np.full(pieces[-1]["W"], -1),
                           roww=np.zeros(pieces[-1]["W"]), dummy=True))
    geom = []
    for r0 in range(0, len(pieces), NCORES):
        rnd = pieces[r0:r0 + NCORES]
        wr = max(p["W"] for p in rnd)
        for p in rnd:
            if p["W"] < wr:
                pad = wr - p["W"]
                p["rows"] = np.concatenate([p["rows"], np.full(pad, -1)])
                p["roww"] = np.concatenate([p["roww"], np.zeros(pad)])
                p["W"] = wr
        geom.append(wr)
    geom = tuple(geom)
    np_ = len(geom)
    mv_tot = sum(geom)
    offs = np.concatenate([[0], np.cumsum(geom)]).astype(int)

    def feat(xT, cols):
        out = np.zeros((D, len(cols)), dtype=ml_dtypes.float8_e4m3)
        real = cols >= 0
        out[:, real] = xT[:, cols[real]]
        return out

    in_maps = []
    piece_meta = []
    for c in range(NCORES):
        mov = np.zeros((D, mv_tot), dtype=ml_dtypes.float8_e4m3)
        lhsA = np.zeros((D, np_ * IT), dtype=ml_dtypes.float8_e4m3)
        mwA = np.zeros((IT, mv_tot), dtype=np.float32)
        meta = []
        for p_i in range(np_):
            p = pieces[p_i * NCORES + c]
            bk, k = p["bk"], p["k"]
            off = offs[p_i]
            if p.get("dummy"):
                meta.append((bk, k, np.zeros(IT)))
                continue
            colT = xsT if bk in ("ss", "st") else xtT
            rowT = xsT if bk in ("ss", "st") else xtT
            if bk == "st":
                rowT = xsT
                colT = xtT
            lhsA[:, p_i * IT:(p_i + 1) * IT] = feat(colT, p["cols"])
            mov[:, off:off + p["W"]] = feat(rowT, p["rows"])
            mwfull = {"ss": mw_ss, "tt": mw_tt, "st": mw_st}[bk]
            rows = p["rows"]
            real = rows >= 0
            mvec = np.zeros(p["W"])
            mvec[real] = mwfull[rows[real]] * p["roww"][real]
            mwA[:, off:off + p["W"]] = mvec[None, :]
            # host-side column weights
            vfull = es if bk == "ss" else ((w * w) * et if bk == "tt" else et)
            cols = p["cols"]
            vcol = np.zeros(IT)
            realc = cols >= 0
            vcol[realc] = vfull[cols[realc]]
            meta.append((bk, k, vcol))
        in_maps.append({
            "mov": mov, "lhsA": lhsA,
            "mw": np.ascontiguousarray(mwA).astype(ml_dtypes.bfloat16),
        })
        piece_meta.append(meta)

    cal = {"ss": 1.0, "tt": 1.0, "st": 1.0}

    global _LAST_GEOM
    _LAST_GEOM = geom
    aux = dict(ms=ms, mt=mt, wt2=wt2, lab=lab, pt=pt, cal=cal,
               piece_meta=piece_meta, geom=geom)
    return in_maps, aux


def _host_finish(results, aux, logits_s):
    ms, mt, wt2 = aux["ms"], aux["mt"], aux["wt2"]
    lab, pt, cal = aux["lab"], aux["pt"], aux["cal"]

    acc = {"ss": 0.0 * np.zeros(C), "tt": np.zeros(C), "st": np.zeros(C)}
    for c, r in enumerate(results):
        L = r["lout"].astype(np.float64)       # [IT, np]
        for p_i, (bk, k, vcol) in enumerate(aux["piece_meta"][c]):
            acc[bk][k] += L[:, p_i] @ vcol

    ss_s = acc["ss"] / cal["ss"]
    ss_t = acc["tt"] / cal["tt"]
    ss_st = acc["st"] / cal["st"]

    n_s = ms.sum(axis=0)
    n_t = mt.sum(axis=0)
    tr_s = n_s
    tr_t = wt2.sum(axis=0)

    def h2(tr, sumsq):
        info = sumsq / (tr + EPS) ** 2
        return -np.log(info + EPS) / LOG2

    h_s = h2(tr_s, ss_s)
    h_t = h2(tr_t, ss_t)
    h_mix = h2(tr_s + tr_t, ss_s + 2.0 * ss_st + ss_t)
    per_class = h_mix - 0.5 * (h_s + h_t)
    valid = (n_s >= 2) & (n_t >= 2)
    n_valid = float(valid.sum())
    creda_sum = float(np.where(valid, per_class, 0.0).sum())
    loss_creda = creda_sum / max(n_valid, 1.0) if n_valid > 0 else 0.0

    zs = np.asarray(logits_s, dtype=np.float64)
    zs = zs - zs.max(axis=1, keepdims=True)
    lse = np.log(np.exp(zs).sum(axis=1))
    logp = zs - lse[:, None]
    loss_cls = -float(np.mean(logp[np.arange(N), lab]))

    loss_ent = -float(np.mean(np.sum(pt * np.log(pt + EPS), axis=1)))

    total = loss_cls + LAMBDA_CREDA * loss_creda + LAMBDA_ENTROPY * loss_ent
    return np.array(total, dtype=np.float32)


def run(inputs, trace=False, repeat=1):
    """Full pipeline; returns (loss, BassKernelResults)."""
    in_maps, aux = _host_prep(**inputs)
    nc = _get_nc(repeat, geom=aux["geom"])
    res = run_bass_kernel_spmd(
        nc, in_maps, core_ids=list(range(NCORES)), trace=trace,
    )
    loss = _host_finish(res.results, aux, inputs["logits_s"])
    return loss, res


def kernel(**inputs) -> np.ndarray:
    loss, _ = run(inputs, trace=False)
    return loss


# revision 26
# speedup vs baseline: 5.9420x; 1.0604x over previous
"""CREDA loss kernel for Trainium2 (8 NeuronCores, SPMD, class-blocked).

Math: the loss needs only K^2 entries: with f = 2/(2*sigma^2+EPS),
K2[i,j] = exp(2f*G[i,j]) * exp(-f*r[i]) * exp(-f*r[j]), G = X @ Y.T, and
every per-class reduction is a quadratic form u^T K2 v.  The device computes,
for a tile of columns j and a window of rows i,
    L[j] = sum_i exp(2f*G[j,i]) * mw[i],
with mw[i] = u[i] * wt[i] * exp(-f*r[i]) (class mask, symmetry doubling
weight, and row norm factor folded into one bf16 vector).  The host applies
the column factors v[j] (mask * exp(-f*r[j])).

Engine mapping: Gram on PE (fp8 DoubleRow, K=1024, columns are the PSUM
partition dim) -> exp on ScalarE (scale=2f, bf16 output) -> weighted row-sum on DVE
(scalar_tensor_tensor accum, the ONLY consumer) -> one [128, npieces] DMA.
The PE runs nothing but identical DoubleRow matmuls - no mode switches, no
PSUM reduce tiles.

Class blocking: only SAME-class (i,j) pairs contribute (the masks are
one-hot).  Rows/cols are sorted by class (labels for fs, pseudo-labels for
ft) and padded per class to nch_k*128.  K_ss/K_tt are symmetric: a wrapped
round-robin covers every unordered 128-chunk pair once (cross-chunk rows get
doubling weight 2, the self-chunk weight 1, inside mw).  K_st is full per
class with weight 1 (the 2x in h_mix stays on host).

A device piece = [128 class-block columns] x [their full row window].
Pieces are sorted by window width, grouped into rounds of 8 (one piece per
core), each round padded to a common width, so all cores run the identical
program (SPMD); block/class/chunk identity lives in host-packed data.
Padded rows carry mw=0; padded/dummy columns get host weight 0.
"""

import numpy as np
import ml_dtypes

import concourse.bacc as bacc
import concourse.tile as tile
import concourse.mybir as mybir
from concourse.bass_utils import run_bass_kernel_spmd

# Problem constants (hardcoded per harness contract)
N = 4096            # N_S == N_T
D = 1024
C = 4
SIGMA = 32.0
EPS = 1e-8
LOG2 = float(np.log(2.0))
LAMBDA_CREDA = 1.0
LAMBDA_ENTROPY = 0.1

NCORES = 8
IT = 128            # columns per piece (PSUM partition dim)
KC = 128            # contraction chunk (PE partition dim)
N_K = D // KC       # 8
WCAP = 1536          # max row-window per piece (PSUM: [128, WCAP] fp32 = 3 banks)

F_SCALE = 2.0 / (2.0 * SIGMA * SIGMA + EPS)
ACT_SCALE = float(2.0 * F_SCALE)

BF16 = mybir.dt.bfloat16
FP32 = mybir.dt.float32
FP8 = mybir.dt.float8e4

_COMPILED = {}
_LAST_GEOM = None


def _build(geom, repeat=1, gp_bufs=2, st_bufs=2, pipe=1, stag=True,
           hints=()):
    """geom: tuple of round widths (one piece of that width per core/round)."""
    widths = list(geom)
    np_ = len(widths)
    wmax = max(widths)
    mv_tot = sum(widths)
    offs = np.concatenate([[0], np.cumsum(widths)]).astype(int)

    nc = bacc.Bacc("TRN2", target_bir_lowering=False, debug=False)
    mov = nc.dram_tensor("mov", [D, mv_tot], FP8, kind="ExternalInput")
    lhsA = nc.dram_tensor("lhsA", [D, np_ * IT], FP8, kind="ExternalInput")
    mw = nc.dram_tensor("mw", [IT, mv_tot], BF16, kind="ExternalInput")
    lout = nc.dram_tensor("lout", [IT, np_], FP32, kind="ExternalOutput")

    with tile.TileContext(nc) as tc:
        with (
            tc.tile_pool(name="const", bufs=1) as const,
            tc.tile_pool(name="ep", bufs=6) as epp,
            tc.tile_pool(name="stage", bufs=st_bufs) as stp,
            tc.tile_pool(name="gps", bufs=2, space="PSUM") as gps,
        ):
            lhsT = const.tile([KC, N_K, np_ * IT], FP8, tag="lhs")
            nc.sync.dma_start(out=lhsT, in_=lhsA.ap().rearrange("(k p) i -> p k i", p=KC))
            mvT = const.tile([KC, N_K, mv_tot], FP8, tag="mov")
            nc.sync.dma_start(out=mvT, in_=mov.ap().rearrange("(k p) j -> p k j", p=KC))
            mwT = const.tile([IT, mv_tot], BF16, tag="mw")
            nc.sync.dma_start(out=mwT, in_=mw.ap())

            def body():
                stage = stp.tile([IT, np_], FP32, tag="stage", bufs=st_bufs,
                                 name="stage")
                for p in range(np_):
                    w, off = widths[p], offs[p]
                    ep = epp.tile([IT, wmax], BF16, tag=f"ep{p % 3}", bufs=2,
                                  name=f"ep_{p}")
                    gp = gps.tile([IT, wmax], FP32, tag="gp", bufs=gp_bufs)
                    for a in range(0, w, 512):
                        b = min(a + 512, w)
                        for k2 in range(N_K // 2):
                            nc.tensor.matmul(
                                gp[:, a:b],
                                lhsT[:, 2 * k2:2 * k2 + 2, p * IT:(p + 1) * IT],
                                mvT[:, 2 * k2:2 * k2 + 2, off + a:off + b],
                                start=(k2 == 0), stop=(k2 == N_K // 2 - 1),
                                perf_mode=mybir.MatmulPerfMode.DoubleRow,
                            )
                    nc.scalar.activation(
                        ep[:, 0:w], gp[:, 0:w],
                        mybir.ActivationFunctionType.Exp,
                        scale=ACT_SCALE,
                    )
                    sc = stp.tile([IT, wmax], BF16, tag=f"sc{p % 2}", bufs=2,
                                  name=f"sc_{p}")
                    nc.vector.scalar_tensor_tensor(
                        out=sc[:, 0:w],
                        in0=ep[:, 0:w],
                        scalar=1.0,
                        in1=mwT[:, off:off + w],
                        op0=mybir.AluOpType.mult,
                        op1=mybir.AluOpType.mult,
                        accum_out=stage[:, p:p + 1],
                    )
                nc.sync.dma_start(out=lout.ap(), in_=stage)

            if repeat == 1:
                body()
            else:
                with tc.For_i(0, repeat, 1, staggered_reset=stag,
                              hint_engines=tuple(hints)):
                    body()

    nc.compile()
    return nc


def _get_nc(repeat=1, geom=None):
    if geom is None:
        geom = _LAST_GEOM
    key = (tuple(geom), repeat)
    if key not in _COMPILED:
        _COMPILED[key] = _build(geom, repeat=repeat)
    return _COMPILED[key]


def _class_index(classes):
    """idx[k] = padded row-index array (len nch_k*128, -1 = pad)."""
    order = np.argsort(classes, kind="stable")
    out = []
    for k in range(C):
        rows = order[classes[order] == k]
        nch = max(1, (len(rows) + IT - 1) // IT)
        idx = np.full(nch * IT, -1, dtype=np.int64)
        idx[:len(rows)] = rows
        out.append(idx)
    return out


def _row_windows(nch):
    """For each col chunk j: list of (row chunk r, doubling weight).

    Chunk r covers col chunks r..r+u_r-1 (wrapped); the transposed view
    gives, per column chunk j, the set of covering row chunks.  Weight 2
    for cross chunks (computed once, counted twice), 1 for the self chunk.
    """
    wins = [[] for _ in range(nch)]
    for r in range(nch):
        if nch % 2 == 1:
            u = (nch + 1) // 2
        else:
            u = nch // 2 + 1 if r < nch // 2 else nch // 2
        for d in range(u):
            j = (r + d) % nch
            wins[j].append((r, 1.0 if d == 0 else 2.0))
    return wins


def _host_prep(features_s, logits_s, features_t, logits_t, labels_s):
    fs = np.asarray(features_s, dtype=np.float32)
    ft = np.asarray(features_t, dtype=np.float32)
    lt = np.asarray(logits_t, dtype=np.float32)
    lab = np.asarray(labels_s).astype(np.int64)

    z = lt.astype(np.float64)
    z = z - z.max(axis=1, keepdims=True)
    pt = np.exp(z)
    pt /= pt.sum(axis=1, keepdims=True)
    pseudo = np.argmax(pt, axis=1)
    h2p = -np.log(np.sum(pt * pt, axis=1) + EPS) / LOG2
    h2max = np.log(float(C)) / LOG2
    w = 1.0 - h2p / (h2max + EPS)

    ms = np.zeros((N, C), dtype=np.float64)
    ms[np.arange(N), lab] = 1.0
    mt = np.zeros((N, C), dtype=np.float64)
    mt[np.arange(N), pseudo] = 1.0
    wt2 = mt * (w * w)[:, None]

    rs = np.sum(fs.astype(np.float64) ** 2, axis=1)
    rt = np.sum(ft.astype(np.float64) ** 2, axis=1)
    es = np.exp(-F_SCALE * rs)
    et = np.exp(-F_SCALE * rt)

    xsT = np.ascontiguousarray(fs.T).astype(ml_dtypes.float8_e4m3)
    xtT = np.ascontiguousarray(ft.T).astype(ml_dtypes.float8_e4m3)

    idx_s = _class_index(lab)
    idx_t = _class_index(pseudo)

    # row-weight vectors in global row order (mask is implicit: rows of the
    # right class only enter via the class-sorted index arrays)
    mw_ss = es.copy()                    # u=1 on real rows
    mw_tt = (w * w) * et
    mw_st = es.copy()

    # ---- build transposed pieces: 128 cols x row window ------------------
    # piece = (W, cols[128], rowidx[W], roww[W], bk, k)
    pieces = []

    def add_piece(bk, k, cols, rowidx, roww):
        pieces.append(dict(W=len(rowidx), bk=bk, k=k, cols=cols,
                           rows=np.asarray(rowidx), roww=np.asarray(roww)))

    for bk, idx in (("ss", idx_s), ("tt", idx_t)):
        for k in range(C):
            nch = len(idx[k]) // IT
            wins = _row_windows(nch)
            for j in range(nch):
                cols = idx[k][j * IT:(j + 1) * IT]
                rowidx, roww = [], []
                for (r, wt) in wins[j]:
                    rowidx.append(idx[k][r * IT:(r + 1) * IT])
                    roww.append(np.full(IT, wt))
                add_piece(bk, k, cols, np.concatenate(rowidx),
                          np.concatenate(roww))
    for k in range(C):
        nch_t = len(idx_t[k]) // IT
        for j in range(nch_t):
            cols = idx_t[k][j * IT:(j + 1) * IT]
            add_piece("st", k, cols, idx_s[k], np.ones(len(idx_s[k])))

    # ---- split row windows larger than WCAP (partial sums add on host) ---
    split = []
    for p in pieces:
        if p["W"] <= WCAP:
            split.append(p)
            continue
        for a in range(0, p["W"], WCAP):
            b = min(a + WCAP, p["W"])
            split.append(dict(W=b - a, bk=p["bk"], k=p["k"], cols=p["cols"],
                              rows=p["rows"][a:b], roww=p["roww"][a:b]))
    pieces = split

    # ---- rounds of 8: sort desc by W, pad round members to round max -----
    pieces.sort(key=lambda p: -p["W"])
    while len(pieces) % NCORES:
        pieces.append(dict(W=pieces[-1]["W"], bk="ss", k=0,
                           cols=np.full(IT, -1),
                           rows=np.full(pieces[-1]["W"], -1),
                           roww=np.zeros(pieces[-1]["W"]), dummy=True))
    geom = []
    for r0 in range(0, len(pieces), NCORES):
        rnd = pieces[r0:r0 + NCORES]
        wr = max(p["W"] for p in rnd)
        for p in rnd:
            if p["W"] < wr:
                pad = wr - p["W"]
                p["rows"] = np.concatenate([p["rows"], np.full(pad, -1)])
                p["roww"] = np.concatenate([p["roww"], np.zeros(pad)])
                p["W"] = wr
        geom.append(wr)
    geom = tuple(geom)
    np_ = len(geom)
    mv_tot = sum(geom)
    offs = np.concatenate([[0], np.cumsum(geom)]).astype(int)

    def feat(xT, cols):
        out = np.zeros((D, len(cols)), dtype=ml_dtypes.float8_e4m3)
        real = cols >= 0
        out[:, real] = xT[:, cols[real]]
        return out

    in_maps = []
    piece_meta = []
    for c in range(NCORES):
        mov = np.zeros((D, mv_tot), dtype=ml_dtypes.float8_e4m3)
        lhsA = np.zeros((D, np_ * IT), dtype=ml_dtypes.float8_e4m3)
        mwA = np.zeros((IT, mv_tot), dtype=np.float32)
        meta = []
        for p_i in range(np_):
            p = pieces[p_i * NCORES + c]
            bk, k = p["bk"], p["k"]
            off = offs[p_i]
            if p.get("dummy"):
                meta.append((bk, k, np.zeros(IT)))
                continue
            colT = xsT if bk in ("ss", "st") else xtT
            rowT = xsT if bk in ("ss", "st") else xtT
            if bk == "st":
                rowT = xsT
                colT = xtT
            lhsA[:, p_i * IT:(p_i + 1) * IT] = feat(colT, p["cols"])
            mov[:, off:off + p["W"]] = feat(rowT, p["rows"])
            mwfull = {"ss": mw_ss, "tt": mw_tt, "st": mw_st}[bk]
            rows = p["rows"]
            real = rows >= 0
            mvec = np.zeros(p["W"])
            mvec[real] = mwfull[rows[real]] * p["roww"][real]
            mwA[:, off:off + p["W"]] = mvec[None, :]
            # host-side column weights
            vfull = es if bk == "ss" else ((w * w) * et if bk == "tt" else et)
            cols = p["cols"]
            vcol = np.zeros(IT)
            realc = cols >= 0
            vcol[realc] = vfull[cols[realc]]
            meta.append((bk, k, vcol))
        in_maps.append({
            "mov": mov, "lhsA": lhsA,
            "mw": np.ascontiguousarray(mwA).astype(ml_dtypes.bfloat16),
        })
        piece_meta.append(meta)

    cal = {"ss": 1.0, "tt": 1.0, "st": 1.0}

    global _LAST_GEOM
    _LAST_GEOM = geom
    aux = dict(ms=ms, mt=mt, wt2=wt2, lab=lab, pt=pt, cal=cal,
               piece_meta=piece_meta, geom=geom)
    return in_maps, aux


def _host_finish(results, aux, logits_s):
    ms, mt, wt2 = aux["ms"], aux["mt"], aux["wt2"]
    lab, pt, cal = aux["lab"], aux["pt"], aux["cal"]

    acc = {"ss": 0.0 * np.zeros(C), "tt": np.zeros(C), "st": np.zeros(C)}
    for c, r in enumerate(results):
        L = r["lout"].astype(np.float64)       # [IT, np]
        for p_i, (bk, k, vcol) in enumerate(aux["piece_meta"][c]):
            acc[bk][k] += L[:, p_i] @ vcol

    ss_s = acc["ss"] / cal["ss"]
    ss_t = acc["tt"] / cal["tt"]
    ss_st = acc["st"] / cal["st"]

    n_s = ms.sum(axis=0)
    n_t = mt.sum(axis=0)
    tr_s = n_s
    tr_t = wt2.sum(axis=0)

    def h2(tr, sumsq):
        info = sumsq / (tr + EPS) ** 2
        return -np.log(info + EPS) / LOG2

    h_s = h2(tr_s, ss_s)
    h_t = h2(tr_t, ss_t)
    h_mix = h2(tr_s + tr_t, ss_s + 2.0 * ss_st + ss_t)
    per_class = h_mix - 0.5 * (h_s + h_t)
    valid = (n_s >= 2) & (n_t >= 2)
    n_valid = float(valid.sum())
    creda_sum = float(np.where(valid, per_class, 0.0).sum())
    loss_creda = creda_sum / max(n_valid, 1.0) if n_valid > 0 else 0.0

    zs = np.asarray(logits_s, dtype=np.float64)
    zs = zs - zs.max(axis=1, keepdims=True)
    lse = np.log(np.exp(zs).sum(axis=1))
    logp = zs - lse[:, None]
    loss_cls = -float(np.mean(logp[np.arange(N), lab]))

    loss_ent = -float(np.mean(np.sum(pt * np.log(pt + EPS), axis=1)))

    total = loss_cls + LAMBDA_CREDA * loss_creda + LAMBDA_ENTROPY * loss_ent
    return np.array(total, dtype=np.float32)


def run(inputs, trace=False, repeat=1):
    """Full pipeline; returns (loss, BassKernelResults)."""
    in_maps, aux = _host_prep(**inputs)
    nc = _get_nc(repeat, geom=aux["geom"])
    res = run_bass_kernel_spmd(
        nc, in_maps, core_ids=list(range(NCORES)), trace=trace,
    )
    loss = _host_finish(res.results, aux, inputs["logits_s"])
    return loss, res


def kernel(**inputs) -> np.ndarray:
    loss, _ = run(inputs, trace=False)
    return loss
